# revision 1
# baseline (speedup 1.0000x reference)
"""Trainium2 Bass kernel for graph-contrastive loss (nn_PrePrompt_75496935129282).

Computation (reference):
    self = segment_sum(logits_origin, ori_idx, G)       # [G, D]
    pos  = segment_sum(logits_pos,  pos_idx, G)         # [G, D]
    sim[g, k]  = cos(self[g], pos[k])   (eps-guarded norms)
    res[g] = log(sum_s exp(sim[g, neg_idx[g, s]])) - sim[g, g]
    out = mean(res)

Device strategy (8 NeuronCores, SPMD):
  - Nodes sharded 8 ways. Host orders each core's nodes by graph block
    (gid >> 7, 16 blocks of 128) with data-driven per-block chunk
    counts: every 128-node chunk targets one block, so the one-hot
    matmul is [128, 128] per chunk. Blocks are processed EVENS FIRST
    so PSUM/stage halves split by block parity.
  - Streams are fp8e4m3; chunk PAIRS go through one DoubleRow matmul
    (256 nodes per PE pass at 0.5 cycles/row); odd tails use a single
    fp8 matmul. One-hots for 16 chunks are generated with a single
    broadcast is_equal against a block-relative bf16 iota.
  - Collectives (bf16, all on one stream): a dummy AllGather at t=0
    absorbs the core-skew barrier; one ReduceScatter gives each core
    its local blocks {2r, 2r+1} of pos; the raw local slice is
    immediately AllGathered (normalize happens after the gather, on
    the tail); self uses two half ReduceScatters (even then odd
    blocks) so the odd half lands right at phase end.
  - Tail: normalize gathered pos table (16 blocks), PE-transpose to
    [d, g], cosine Gram of the core's 256 self rows vs all 2048
    columns, denominator = exp-accumulate of (sim + ln(count)) with a
    host-precomputed f32 count table, numerator from raw local dots
    scaled by inverse norms. Per-core losses summed on host.
"""

import os
import sys

sys.path.insert(0, "/opt/trn_rl_repo")

import numpy as np

import concourse.bacc as bacc
import concourse.bass as bass  # noqa: F401
import concourse.mybir as mybir
import concourse.tile as tile
from concourse import bass_isa
from concourse.bass_utils import run_bass_kernel_spmd


def _ensure_ntff_hook():
    """The agent image's antenv lacks axon_hooks; inject it and register
    the ctypes NTFF profiling hook so trace=True works under axon."""
    import types

    import antenv

    if hasattr(antenv, "axon_hooks"):
        return
    mod = types.ModuleType("antenv.axon_hooks")
    mod._hook = None

    def set_axon_ntff_profile_hook(h):
        mod._hook = h

    def get_axon_ntff_profile_hook():
        return mod._hook

    mod.set_axon_ntff_profile_hook = set_axon_ntff_profile_hook
    mod.get_axon_ntff_profile_hook = get_axon_ntff_profile_hook
    sys.modules["antenv.axon_hooks"] = mod
    antenv.axon_hooks = mod
    try:
        from trn_agent_boot.trn_boot import _ntff_profile_via_ctypes

        mod._hook = _ntff_profile_via_ctypes("/opt/axon/libaxon_pjrt.so")
    except Exception as e:  # pragma: no cover
        print(f"ntff hook registration failed: {e}")


F32 = mybir.dt.float32
BF16 = mybir.dt.bfloat16
F8 = mybir.dt.float8e4

G = 2048
S = 127
D = 256
NCORES = 8
P = 128
A = 16  # chunk-count quantum (padding granularity)
AW = 32  # chunks per DMA group (packet size = AW*D fp8 per partition)
NBUK = 16  # graph blocks of 128
GLOC = G // NCORES  # 256

# even blocks first, then odd: PSUM generation k holds blocks ORDER[8k:8k+8]
ORDER = list(range(0, NBUK, 2)) + list(range(1, NBUK, 2))

_MM_RAW = os.environ.get("KERNEL_MM_DT", "f8")
MMDT = BF16 if _MM_RAW == "bf16" else F8


def _chunk_groups(nchunk):
    """DMA groups: small warm-up groups (fast pipeline start), then
    AW-wide groups plus a ragged tail."""
    out = []
    base = 0
    for w in (8, 8, 16):
        if base + w <= nchunk:
            out.append((base, w))
            base += w
    while base < nchunk:
        w = min(AW, nchunk - base)
        out.append((base, w))
        base += w
    return out


def _plan_units(cb):
    """Walk chunks in even-first block order; greedily pair same-bucket
    chunks that sit in the same DMA group (DoubleRow), singles otherwise.

    Returns (units, half_end_unit, nchunk) with units = [(bucket, c0, k)]."""
    nchunk = sum(cb)
    ends = {base + w - 1 for base, w in _chunk_groups(nchunk)}
    units = []
    c = 0
    for b in ORDER:
        rem = cb[b]
        while rem:
            if rem >= 2 and c not in ends and MMDT is F8:
                units.append((b, c, 2))
                c += 2
                rem -= 2
            else:
                units.append((b, c, 1))
                c += 1
                rem -= 1
    assert c == nchunk
    first8 = set(ORDER[:8])
    half_end_unit = max(i for i, u in enumerate(units) if u[0] in first8)
    return units, half_end_unit, nchunk


def build_nc(cb):
    """SPMD Bass program; cb[b] = chunks assigned to graph block b."""
    nchunk = sum(cb)
    assert nchunk % A == 0
    groups32 = _chunk_groups(nchunk)
    ngrp = len(groups32)
    units, half_end_unit, nck = _plan_units(cb)
    assert nck == nchunk
    first_unit = {}
    last_unit = {}
    for i, (b, _, _) in enumerate(units):
        first_unit.setdefault(b, i)
        last_unit[b] = i
    # units grouped by DMA group
    grp_of = {}
    for gi, (base, w) in enumerate(groups32):
        for c in range(base, base + w):
            grp_of[c] = gi
    sup_units = [[] for _ in range(ngrp)]
    for i, (b, c0, k) in enumerate(units):
        sup_units[grp_of[c0]].append((i, b, c0, k))
    slot = {b: ORDER.index(b) % 8 for b in range(NBUK)}
    DBG = os.environ.get("KERNEL_DEBUG", "0") == "1"

    nc = bacc.Bacc(
        "TRN2",
        target_bir_lowering=False,
        debug=False,
        num_devices=NCORES,
    )
    groups = [list(range(NCORES))]
    EQ = mybir.AluOpType.is_equal
    ADD = mybir.AluOpType.add
    MUL = mybir.AluOpType.mult
    SUB = mybir.AluOpType.subtract
    AF = mybir.ActivationFunctionType

    # ---- I/O ----
    xp_d = nc.dram_tensor("xp", [nchunk * P * D], MMDT, kind="ExternalInput").ap()
    xo_d = nc.dram_tensor("xo", [nchunk * P * D], MMDT, kind="ExternalInput").ap()
    idx_d = nc.dram_tensor("idx", [P, 2, nchunk], BF16, kind="ExternalInput").ap()
    lnc_d = nc.dram_tensor("lnc", [P, 2, G], F32, kind="ExternalInput").ap()
    loss_out = nc.dram_tensor("loss", [1, 1], F32, kind="ExternalOutput").ap()
    if DBG:
        dbg_pa = nc.dram_tensor(
            "dbg_pa", [P, 2, NCORES, D], F8, kind="ExternalOutput"
        ).ap()
        dbg_sim0 = nc.dram_tensor("dbg_sim0", [P, 1], F32, kind="ExternalOutput").ap()
        dbg_den = nc.dram_tensor("dbg_den", [P, 2], F32, kind="ExternalOutput").ap()

    # ---- internal DRAM ----
    p_stage = nc.dram_tensor("p_stage", [NBUK, P, D], F8).ap()
    s_stage = nc.dram_tensor("s_stage", [2, 8, P, D], F8).ap()
    p_loc = nc.dram_tensor("p_loc", [2, P, D], F8).ap()
    s_loc = nc.dram_tensor("s_loc", [2, P, D], F8).ap()
    phat_all = nc.dram_tensor(
        "phat_all", [NCORES, 2, P, D], F8, addr_space="Shared"
    ).ap()


    with tile.TileContext(nc) as tc:
        with (
            tc.tile_pool(name="const", bufs=1) as cpool,
            tc.tile_pool(name="big", bufs=1) as big,
        ):
            # ---- constants / one-shot loads ----
            iota_i = cpool.tile([P, P], mybir.dt.int32, tag="iota_i")
            nc.gpsimd.iota(iota_i[:], pattern=[[1, P]], base=0, channel_multiplier=0)
            iota_bf = cpool.tile([P, P], BF16, tag="iota_bf")
            nc.vector.tensor_copy(iota_bf[:], iota_i[:])
            iota_1 = iota_bf[:].rearrange("p (o x) -> p o x", o=1)
            eps_col = cpool.tile([P, 1], F32, tag="eps_col")
            nc.vector.memset(eps_col[:], 1e-16)
            from concourse.masks import make_identity

            ident_f = cpool.tile([P, P], F32, tag="ident_f")
            make_identity(nc, ident_f[:])
            ident = cpool.tile([P, P], BF16, tag="ident")
            nc.vector.tensor_copy(ident[:], ident_f[:])

            it_sb = cpool.tile([P, 2, nchunk], BF16, tag="it")
            nc.sync.dma_start(out=it_sb[:], in_=idx_d)
            lnc_sb = big.tile([P, 2, G], F32, tag="lnc")
            nc.scalar.dma_start(out=lnc_sb[:], in_=lnc_d)

            # ============ segment-sum phase ============
            def seg_phase(x_d, t_row, stage_half_ap, psum_bufs, tag, on_half, on_full):
                """fp8 DoubleRow bucketed matmuls -> bf16 stage halves.

                stage_half_ap(gen) -> [8, P, D] dram AP for that parity."""
                with (
                    tc.tile_pool(name=f"ps_{tag}", bufs=psum_bufs, space="PSUM") as pseg,
                    tc.tile_pool(name=f"st_{tag}", bufs=3) as stream,
                    tc.tile_pool(name=f"oh_{tag}", bufs=3) as ohp,
                    tc.tile_pool(name=f"sb_{tag}", bufs=2) as segsb,
                ):
                    acc = pseg.tile([P, 8, D], F32, tag="acc")
                    for gi, (base, w) in enumerate(groups32):
                        xt = stream.tile([P, w, D], MMDT, tag=f"xt{w}")
                        nc.sync.dma_start(
                            out=xt[:],
                            in_=x_d[base * P * D : (base + w) * P * D].rearrange(
                                "(p a d) -> p a d", p=P, a=w, d=D
                            ),
                        )
                        ohm = ohp.tile([P, w, P], MMDT, tag=f"ohm{w}")
                        it_b = (
                            it_sb[:, t_row, base : base + w]
                            .rearrange("p (a o) -> p a o", o=1)
                            .broadcast_to([P, w, P])
                        )
                        nc.vector.tensor_tensor(
                            out=ohm[:], in0=iota_1.broadcast_to([P, w, P]),
                            in1=it_b, op=EQ,
                        )
                        for i, b, c0, k in sup_units[gi]:
                            a = c0 - base
                            if k == 2:
                                nc.tensor.matmul(
                                    out=acc[:, slot[b], :],
                                    lhsT=ohm[:, a : a + 2, :],
                                    rhs=xt[:, a : a + 2, :],
                                    start=(i == first_unit[b]),
                                    stop=(i == last_unit[b]),
                                    perf_mode=mybir.MatmulPerfMode.DoubleRow,
                                )
                            else:
                                nc.tensor.matmul(
                                    out=acc[:, slot[b], :],
                                    lhsT=ohm[:, a, :],
                                    rhs=xt[:, a, :],
                                    start=(i == first_unit[b]),
                                    stop=(i == last_unit[b]),
                                )
                            if i == half_end_unit:
                                sbh = segsb.tile([P, 8, D], F8, tag="sbh")
                                nc.scalar.copy(sbh[:], acc[:])
                                nc.scalar.dma_start(
                                    out=stage_half_ap(0).rearrange("h p d -> p h d"),
                                    in_=sbh[:],
                                )
                                on_half()
                                acc = pseg.tile([P, 8, D], F32, tag="acc")
                    sbh = segsb.tile([P, 8, D], F8, tag="sbh")
                    nc.scalar.copy(sbh[:], acc[:])
                    nc.scalar.dma_start(
                        out=stage_half_ap(1).rearrange("h p d -> p h d"), in_=sbh[:]
                    )
                    on_full()

            # ---- phase P (pos): single RS then single AG of the raw slice ----
            p_stage_par = p_stage.rearrange("(b t) p d -> t b p d", t=2)
            pa_hat = big.tile([P, 2, NCORES, D], F8, tag="pa_hat")

            def p_done():
                nc.gpsimd.collective_compute(
                    "ReduceScatter",
                    ADD,
                    replica_groups=groups,
                    ins=[p_stage[:]],
                    outs=[p_loc[:]],
                )
                nc.gpsimd.collective_compute(
                    "AllGather",
                    mybir.AluOpType.bypass,
                    replica_groups=groups,
                    ins=[p_loc[:]],
                    outs=[phat_all[:]],
                )
                for l in range(2):
                    nc.gpsimd.dma_start(
                        out=pa_hat[:, l],
                        in_=phat_all[:, l].rearrange("c g d -> g c d"),
                    )

            seg_phase(
                xp_d, 0, lambda gen: p_stage_par[gen], 2, "p",
                on_half=lambda: None,
                on_full=p_done,
            )
            # raw local pos slice (for the numerator), loaded off-stream
            pl_sb = big.tile([P, 2, D], F8, tag="pl")
            nc.gpsimd.dma_start(out=pl_sb[:], in_=p_loc.rearrange("l p d -> p l d"))

            # ---- phase O (origin/self): per-parity RS ----

            def s_par(par):
                nc.gpsimd.collective_compute(
                    "ReduceScatter",
                    ADD,
                    replica_groups=groups,
                    ins=[s_stage[par]],
                    outs=[s_loc[par : par + 1]],
                )

            seg_phase(
                xo_d, 1, lambda gen: s_stage[gen], 1, "o",
                on_half=lambda: s_par(0),
                on_full=lambda: s_par(1),
            )

            if DBG:
                nc.sync.dma_start(out=dbg_pa, in_=pa_hat[:])

            # ================= tail =================
            # inverse norms for the raw local pos slice (numerator)
            sqp = big.tile([P, 2, D], F32, tag="sqp")
            nc.vector.tensor_tensor(out=sqp[:], in0=pl_sb[:], in1=pl_sb[:], op=MUL)
            n2p = big.tile([P, 2], F32, tag="n2p")
            nc.vector.tensor_reduce(
                out=n2p[:], in_=sqp[:], axis=mybir.AxisListType.X, op=ADD
            )
            invp_r = big.tile([P, 2], F32, tag="invp_r")
            nc.vector.tensor_scalar(
                out=invp_r[:], in0=n2p[:], scalar1=1e-16, scalar2=None, op0=ADD
            )
            nc.vector.reciprocal_approx_fast(invp_r[:], invp_r[:])
            invp = big.tile([P, 2], F32, tag="invp")
            nc.scalar.activation(out=invp[:], in_=invp_r[:], func=AF.Sqrt)

            # self local slice tiles (halves processed as RS results land)
            sl_sb = big.tile([P, 2, D], F8, tag="sl")
            sqs = big.tile([P, 2, D], F32, tag="sqs")
            n2s = big.tile([P, 2], F32, tag="n2s")
            invs_r = big.tile([P, 2], F32, tag="invs_r")
            invs = big.tile([P, 2], F32, tag="invs")
            shat = big.tile([P, 2, D], BF16, tag="shat")

            def s_half(lo):
                nc.gpsimd.dma_start(
                    out=sl_sb[:, lo, :],
                    in_=s_loc[lo : lo + 1].rearrange("o p d -> p (o d)"),
                )
                nc.vector.tensor_tensor(
                    out=sqs[:, lo, :], in0=sl_sb[:, lo, :], in1=sl_sb[:, lo, :],
                    op=MUL,
                )
                nc.vector.tensor_reduce(
                    out=n2s[:, lo : lo + 1],
                    in_=sqs[:, lo : lo + 1, :],
                    axis=mybir.AxisListType.X,
                    op=ADD,
                )
                nc.vector.tensor_scalar(
                    out=invs_r[:, lo : lo + 1],
                    in0=n2s[:, lo : lo + 1],
                    scalar1=1e-16,
                    scalar2=None,
                    op0=ADD,
                )
                nc.vector.reciprocal_approx_fast(
                    invs_r[:, lo : lo + 1], invs_r[:, lo : lo + 1]
                )
                nc.scalar.activation(
                    out=invs[:, lo : lo + 1],
                    in_=invs_r[:, lo : lo + 1],
                    func=AF.Sqrt,
                )
                nc.vector.tensor_scalar(
                    out=shat[:, lo, :],
                    in0=sl_sb[:, lo, :],
                    scalar1=invs[:, lo : lo + 1],
                    scalar2=None,
                    op0=MUL,
                )

            # per-parity: normalize gathered pos blocks + transpose
            # pn_T columns are PARITY-MAJOR: col-block k holds block ORDER[k]
            sq16 = big.tile([P, 2, NCORES, D], F32, tag="sq16")
            n16 = big.tile([P, 2, NCORES], F32, tag="n16")
            inv16_r = big.tile([P, 2, NCORES], F32, tag="inv16_r")
            inv16 = big.tile([P, 2, NCORES], F32, tag="inv16")
            phn = big.tile([P, 2, NCORES, D], BF16, tag="phn")
            pn_T = big.tile([P, 2, G], F8, tag="pn_T")
            sn_T = big.tile([P, 2, 2 * P], F8, tag="sn_T")
            with tc.tile_pool(name="ps_tr", bufs=4, space="PSUM") as ptr:

                def tr(dst_ap, src_ap, ci):
                    tps = ptr.tile([P, P], BF16, tag="tr")
                    nc.tensor.transpose(out=tps[:], in_=src_ap, identity=ident[:])
                    if ci % 2 == 0:
                        nc.scalar.copy(dst_ap, tps[:])
                    else:
                        nc.vector.tensor_copy(dst_ap, tps[:])

                nc.vector.tensor_tensor(
                    out=sq16[:], in0=pa_hat[:], in1=pa_hat[:], op=MUL
                )
                nc.vector.tensor_reduce(
                    out=n16[:],
                    in_=sq16[:],
                    axis=mybir.AxisListType.X,
                    op=ADD,
                )
                nc.vector.tensor_scalar(
                    out=inv16_r[:], in0=n16[:], scalar1=1e-16,
                    scalar2=None, op0=ADD,
                )
                nc.vector.reciprocal_approx_fast(inv16_r[:], inv16_r[:])
                nc.scalar.activation(out=inv16[:], in_=inv16_r[:], func=AF.Sqrt)
                ci = 0
                for par in range(2):
                    for c in range(NCORES):
                        nc.vector.tensor_scalar(
                            out=phn[:, par, c, :],
                            in0=pa_hat[:, par, c, :],
                            scalar1=inv16[:, par, c : c + 1],
                            scalar2=None,
                            op0=MUL,
                        )
                for par in range(2):
                    for c in range(NCORES):
                        for db in range(2):
                            k = 2 * c + par
                            tr(
                                pn_T[:, db, k * P : (k + 1) * P],
                                phn[:, par, c, db * P : (db + 1) * P],
                                ci,
                            )
                            ci += 1
                for lo in range(2):
                    s_half(lo)
                    for db in range(2):
                        tr(
                            sn_T[:, db, lo * P : (lo + 1) * P],
                            shat[:, lo, db * P : (db + 1) * P],
                            ci,
                        )
                        ci += 1

            # numerator: sim0_total[p] = sum_lo <s_raw, p_raw> * invs * invp
            rd = big.tile([P, 2, D], F32, tag="rd")
            nc.vector.tensor_tensor(out=rd[:], in0=sl_sb[:], in1=pl_sb[:], op=MUL)
            rd2 = big.tile([P, 2], F32, tag="rd2")
            nc.vector.tensor_reduce(
                out=rd2[:], in_=rd[:], axis=mybir.AxisListType.X, op=ADD
            )
            s0a = big.tile([P, 2], F32, tag="s0a")
            nc.vector.tensor_tensor(out=s0a[:], in0=rd2[:], in1=invs[:], op=MUL)
            s0b = big.tile([P, 2], F32, tag="s0b")
            nc.vector.tensor_tensor(out=s0b[:], in0=s0a[:], in1=invp[:], op=MUL)
            sim0 = big.tile([P, 1], F32, tag="sim0")
            nc.vector.tensor_reduce(
                out=sim0[:], in_=s0b[:], axis=mybir.AxisListType.X, op=ADD
            )
            if DBG:
                nc.sync.dma_start(out=dbg_sim0, in_=sim0[:])

            # ---- Gram + loss: per (row-block lo, column-parity) ----
            denp = big.tile([P, 2], F32, tag="denp")
            with (
                tc.tile_pool(name="ps_gram", bufs=2, space="PSUM") as pgram,
                tc.tile_pool(name="gl", bufs=2) as gl,
            ):
                for lo in range(2):
                    pg = pgram.tile([P, 4, 512], F32, tag="pg")
                    for q in range(4):
                        nc.tensor.matmul(
                            out=pg[:, q, :],
                            lhsT=sn_T[:, :, lo * P : (lo + 1) * P],
                            rhs=pn_T[:, :, q * 512 : (q + 1) * 512],
                            start=True,
                            stop=True,
                            perf_mode=mybir.MatmulPerfMode.DoubleRow,
                        )
                    simln = gl.tile([P, G], F32, tag="simln")
                    nc.vector.tensor_tensor(
                        out=simln[:],
                        in0=pg[:].rearrange("p a b -> p (a b)"),
                        in1=lnc_sb[:, lo, :],
                        op=ADD,
                    )
                    ed = gl.tile([P, G], BF16, tag="ed")
                    nc.scalar.activation(
                        out=ed[:],
                        in_=simln[:],
                        func=AF.Exp,
                        accum_out=denp[:, lo : lo + 1],
                    )
            if DBG:
                nc.sync.dma_start(out=dbg_den, in_=denp[:])

            lden2 = big.tile([P, 2], F32, tag="lden2")
            nc.scalar.activation(out=lden2[:], in_=denp[:], func=AF.Ln)
            t0 = big.tile([P, 1], F32, tag="t0")
            nc.vector.tensor_reduce(
                out=t0[:], in_=lden2[:], axis=mybir.AxisListType.X, op=ADD
            )
            t1 = big.tile([P, 1], F32, tag="t1")
            nc.vector.tensor_tensor(out=t1[:], in0=t0[:], in1=sim0[:], op=SUB)
            ones_col = big.tile([P, 1], F32, tag="ones_col")
            nc.vector.memset(ones_col[:], 1.0)
            with tc.tile_pool(name="ps_ls", bufs=1, space="PSUM") as pls:
                lps = pls.tile([1, 1], F32, tag="lps")
                nc.tensor.matmul(
                    out=lps[:], lhsT=t1[:], rhs=ones_col[:], start=True, stop=True
                )
                lsum1 = big.tile([1, 1], F32, tag="lsum1")
                nc.scalar.copy(lsum1[:], lps[:])
            nc.sync.dma_start(out=loss_out[:], in_=lsum1[:])
    nc.compile()
    return nc


def _chunk_plan(idx_list):
    """cb[b] = chunk count covering max bucket occupancy over all
    (core, table) shards; total padded to a multiple of A."""
    maxc = np.zeros(NBUK, np.int64)
    for gids in idx_list:
        cnt = np.bincount((gids >> 7).astype(np.int64), minlength=NBUK)
        maxc = np.maximum(maxc, cnt)
    cb = [max(1, int(np.ceil(c / P))) for c in maxc]
    i = 0
    while sum(cb) % A != 0:
        cb[i % NBUK] += 1
        i += 1
    return cb


def _pack_shard(x, gids, cb, np_mm):
    """Order a core's nodes bucket-major (even blocks first) into the
    padded chunk layout.

    Returns (x_packed [nsup, P, A, D] np_mm, idx_rel [P, nchunk])."""
    nchunk = sum(cb)
    key = (gids >> 7).astype(np.int64)
    counts = np.bincount(key, minlength=NBUK)
    off = {}
    c = 0
    for b in ORDER:
        off[b] = c * P
        c += cb[b]
    pos_in_order = np.asarray([ORDER.index(b) for b in range(NBUK)], np.int64)
    order = np.argsort(pos_in_order[key], kind="stable")
    dst = np.concatenate([off[b] + np.arange(counts[b]) for b in ORDER])
    xpad = np.zeros((nchunk * P, D), np.float32)
    ipad = np.full((nchunk * P,), -1.0, np.float32)
    xpad[dst] = x[order]
    ipad[dst] = (gids[order] & 127).astype(np.float32)
    blocks = []
    for base, w in _chunk_groups(nchunk):
        blk = xpad[base * P : (base + w) * P].reshape(w, P, D).transpose(1, 0, 2)
        blocks.append(blk.reshape(-1))
    x_packed = np.concatenate(blocks).astype(np_mm)
    idx_rel = np.ascontiguousarray(ipad.reshape(nchunk, P).T)
    return x_packed, idx_rel


def _prep_inputs(logits_origin, logits_pos, ori_idx, pos_idx, neg_idx):
    import ml_dtypes  # noqa: F401

    np_mm = np.dtype(mybir.dt.np(MMDT))
    np_bf = np.dtype(mybir.dt.np(BF16))
    xo = np.ascontiguousarray(np.asarray(logits_origin, dtype=np.float32))
    xp = np.ascontiguousarray(np.asarray(logits_pos, dtype=np.float32))
    oi = np.asarray(ori_idx).astype(np.int64)
    pi = np.asarray(pos_idx).astype(np.int64)
    neg = np.asarray(neg_idx)
    n = xo.shape[0]
    assert xo.shape == (n, D) and xp.shape == (n, D)
    assert neg.shape == (G, S)

    nloc = (n + NCORES - 1) // NCORES
    shards = []
    for r in range(NCORES):
        lo = r * nloc
        hi = min(n, lo + nloc)
        shards.append((xo[lo:hi], oi[lo:hi], xp[lo:hi], pi[lo:hi]))
    cb = _chunk_plan([s[1] for s in shards] + [s[3] for s in shards])

    cnt = np.zeros((G, G), dtype=np.float64)
    rows = np.repeat(np.arange(G), S)
    np.add.at(cnt, (rows, neg.ravel().astype(np.int64)), 1.0)
    with np.errstate(divide="ignore"):
        lncnt = np.where(cnt > 0, np.log(cnt), -30000.0).astype(np.float32)

    in_maps = []
    for r in range(NCORES):
        xo_r, oi_r, xp_r, pi_r = shards[r]
        xp_pk, ip_rel = _pack_shard(xp_r, pi_r, cb, np_mm)
        xo_pk, io_rel = _pack_shard(xo_r, oi_r, cb, np_mm)
        idx_pk = np.stack([ip_rel, io_rel], axis=1).astype(np_bf)  # [P, 2, nchunk]
        # local graphs = blocks {2r, 2r+1} = [256r, 256r+256)
        lnc_r = np.stack(
            [lncnt[r * GLOC + lo * P : r * GLOC + (lo + 1) * P] for lo in range(2)],
            axis=1,
        ).astype(np.float32)  # [P, 2, G]
        in_maps.append(
            {
                "xp": xp_pk,
                "xo": xo_pk,
                "idx": np.ascontiguousarray(idx_pk),
                "lnc": np.ascontiguousarray(lnc_r),
            }
        )
    return cb, in_maps


def kernel(
    logits_origin,
    logits_pos,
    ori_idx,
    pos_idx,
    neg_idx,
    _trace=False,
    _tmpdir=None,
):
    cb, in_maps = _prep_inputs(logits_origin, logits_pos, ori_idx, pos_idx, neg_idx)
    if _trace:
        _ensure_ntff_hook()
    nc = build_nc(cb)
    res = run_bass_kernel_spmd(
        nc,
        in_maps,
        core_ids=list(range(NCORES)),
        trace=_trace,
        tmpdir=_tmpdir,
    )
    kernel._last_results = res
    total = sum(float(res.results[r]["loss"][0, 0]) for r in range(NCORES))
    return np.asarray(np.float32(total / G))


kernel._last_results = None


if __name__ == "__main__":
    rng = np.random.default_rng(0)
    n = 4096
    inputs = {
        "logits_origin": rng.standard_normal((n, D), dtype=np.float32),
        "logits_pos": rng.standard_normal((n, D), dtype=np.float32),
        "ori_idx": rng.integers(0, G, n, dtype=np.int32),
        "pos_idx": rng.integers(0, G, n, dtype=np.int32),
        "neg_idx": rng.integers(0, G, (G, S), dtype=np.int32),
    }

    def np_ref(logits_origin, logits_pos, ori_idx, pos_idx, neg_idx):
        x = logits_origin.astype(np.float64)
        y = logits_pos.astype(np.float64)
        self_l = np.zeros((G, D))
        pos_l = np.zeros((G, D))
        np.add.at(self_l, ori_idx, x)
        np.add.at(pos_l, pos_idx, y)
        eps = 1e-8
        na = np.maximum(np.linalg.norm(self_l, axis=1), eps)
        nb = np.maximum(np.linalg.norm(pos_l, axis=1), eps)
        sh = self_l / na[:, None]
        ph = pos_l / nb[:, None]
        gram = sh @ ph.T
        sim0 = np.einsum("gd,gd->g", sh, ph)
        e = np.exp(gram)
        den = np.array([e[g, neg_idx[g]].sum() for g in range(G)])
        res = np.log(den) - sim0
        return res.mean()

    expected = np_ref(**inputs)
    actual = kernel(**inputs)
    err = abs(actual - expected) / max(abs(expected), 1e-12)
    print(f"expected={expected:.6f} actual={float(actual):.6f} relerr={err:.3e}")

